# revision 14
# baseline (speedup 1.0000x reference)
"""Trainium2 Bass kernel for nn_MessageGNN (gnn_message_passing).

Sharding: destination-sharded edges across 8 cores.  Core k owns clauses
[k*50000,(k+1)*50000) and vars [k*12500,(k+1)*12500) plus every edge whose
destination falls in its slice, so segment sums are fully core-local.
All 8 cores run ONE identical Bass program (SPMD); per-core variation
lives entirely in the data.  The edge schedule is made uniform by padding
every 256-destination window's edge count to the max over the 8 cores
(pad slots carry zero x, weight 0, dst sentinel 70000).

Host-side preprocessing folds everything per-edge into one pre-gathered
feature-major payload: x'_e = x_src(e) + ([sat_e, 1] @ [W_sat; b] @
W_emb^{-1}), so the edge MLP is a single f16 matmul z = x' W_emb per
128-edge tile (W_emb is square and well-conditioned; the fold costs
~6e-3 absolute at the edge stage, well under the 2e-2 gate).  A DVE
one-hot (iota==dst)*1/deg [e,256] and one N=256 matmul accumulate each
window's h^T [128d, 256dst] in PSUM; Prelu runs grouped on the Scalar
engine.  The node MLP consumes two windows at a time (N=512): 4 matmuls
(feats+bias / h / host-projected ctx one-hot / emb), outputs leave
d-major [128, Np] and the host transposes.  Phase 3 (context update)
runs on host from the returned node embeddings.
"""

import sys

sys.path.insert(0, "/opt/trn_rl_repo")

import numpy as np

M = 8
WIN = 256
P = 128
GRP = 4
PAD_DST = 70000.0

F16 = np.float16
F32 = np.float32


def _side_prep(src, dst, sat, emb32, WsbInv, n_dst):
    """Sort edges by destination, build the shared padded window schedule
    and the per-core slot arrays (stacked core-major for shard_map).

    x' = emb[src] + [sat, 1] @ WsbInv  (sat/bias folded into emb space)."""
    ndc = n_dst // M
    nwin = -(-ndc // WIN)
    order = np.argsort(dst, kind="stable")
    src_s, dst_s, sat_s = src[order], dst[order], sat[order]
    bounds = np.searchsorted(dst_s, np.arange(M + 1) * ndc)

    cnts = np.zeros((M, nwin), np.int64)
    wlocs = []
    for k in range(M):
        lo, hi = bounds[k], bounds[k + 1]
        wloc = (dst_s[lo:hi] - k * ndc) // WIN
        cnts[k] = np.bincount(wloc, minlength=nwin)
        wlocs.append(wloc)
    npad = np.maximum(((cnts.max(0) + P - 1) // P) * P, P)  # [nwin]
    offs = np.zeros(nwin + 1, np.int64)
    offs[1:] = np.cumsum(npad)
    S = int(offs[-1])
    T = S // P

    wrec = (1.0 / np.maximum(np.bincount(dst, minlength=n_dst), 1.0)).astype(F32)

    xT = np.zeros((M * P, S), F16)
    dstw = np.full((M, S), PAD_DST, F32)
    wsc = np.zeros((M, S), F32)
    for k in range(M):
        lo, hi = bounds[k], bounds[k + 1]
        wloc = wlocs[k]
        runstart = np.zeros(nwin, np.int64)
        runstart[1:] = np.cumsum(cnts[k][:-1])
        n = hi - lo
        pos = offs[wloc] + (np.arange(n) - runstart[wloc])
        xp = emb32[src_s[lo:hi]] + sat_s[lo:hi].astype(F32) @ WsbInv[:4] \
            + WsbInv[4]
        xT[k * P:(k + 1) * P, pos] = xp.astype(F16).T
        dstw[k, pos] = (dst_s[lo:hi] - k * ndc - wloc * WIN).astype(F32)
        wsc[k, pos] = wrec[dst_s[lo:hi]]
    dstT = np.ascontiguousarray(
        dstw.reshape(M, T, P).transpose(0, 2, 1).reshape(M * P, T))
    wT = np.ascontiguousarray(
        wsc.reshape(M, T, P).transpose(0, 2, 1).reshape(M * P, T)).astype(F16)
    return dict(nwin=nwin, npad=npad.tolist(), S=S, T=T,
                xT=xT, dstT=dstT, wT=wT)


def _node_prep(feats, emb16, ctx_ids, n_nodes, Np):
    """Per-core node-phase arrays, stacked core-major."""
    nn = n_nodes // M
    nf = feats.shape[1]
    featsT = np.zeros((M * (nf + 1), Np), F16)
    embT = np.zeros((M * P, Np), F16)
    ohuT = np.zeros((M * 64, Np), F16)
    for k in range(M):
        fs, es, cs_ = (a[k * nn:(k + 1) * nn] for a in (feats, emb16, ctx_ids))
        featsT[k * (nf + 1):k * (nf + 1) + nf, :nn] = fs.T.astype(F16)
        featsT[k * (nf + 1) + nf, :nn] = 1.0
        embT[k * P:(k + 1) * P, :nn] = es.T
        ohuT[k * 64 + cs_, np.arange(nn)] = 1.0
    return featsT, embT, ohuT


def _build_program(meta):
    import concourse.mybir as mybir
    import concourse.tile as tile
    from concourse import bacc

    f16, f32 = mybir.dt.float16, mybir.dt.float32

    nc = bacc.Bacc("TRN2", target_bir_lowering=False, debug=False, num_devices=1)
    io = {}

    def dram(name, shape, dt, kind="ExternalInput"):
        io[name] = nc.dram_tensor(name, list(shape), dt, kind=kind)
        return io[name]

    for side in ("A", "B"):
        pl = meta[side]
        dram(f"xT{side}", [P, pl["S"]], f16)
        dram(f"dstT{side}", [P, pl["T"]], f32)
        dram(f"wT{side}", [P, pl["T"]], f16)
        dram(f"Wemb{side}", [P, P], f16)
    for sd in ("C", "V"):
        Np = meta[f"Np{sd}"]
        dram(f"featsT{sd}", [17, Np], f16)
        dram(f"embT{sd}", [P, Np], f16)
        dram(f"ohuT{sd}", [64, Np], f16)
        dram(f"Wf{sd}", [17, P], f16)
        dram(f"Wh{sd}", [P, P], f16)
        dram(f"We{sd}", [P, P], f16)
        dram(f"ctxproj{sd}", [64, P], f16)
        dram(f"out{sd}", [P, Np], f16, kind="ExternalOutput")

    maxslot = max(max(meta["A"]["npad"]), max(meta["B"]["npad"]))

    with tile.TileContext(nc) as tc:
        with tc.tile_pool(name="const", bufs=1) as cpool, \
             tc.tile_pool(name="xs", bufs=3) as xpool, \
             tc.tile_pool(name="work", bufs=5) as wpool, \
             tc.tile_pool(name="oh", bufs=12) as opool, \
             tc.tile_pool(name="psA", bufs=2, space="PSUM") as psA, \
             tc.tile_pool(name="psH", bufs=2, space="PSUM") as psH, \
             tc.tile_pool(name="psN", bufs=2, space="PSUM") as psN:

            iota_i = cpool.tile([P, WIN], mybir.dt.int32)
            nc.gpsimd.iota(iota_i[:], pattern=[[1, WIN]], base=0,
                           channel_multiplier=0)
            iota16 = cpool.tile([P, WIN], f16)
            nc.vector.tensor_copy(iota16[:], iota_i[:])

            wt = {}
            for nm in ("WembA", "WembB",
                       "WfC", "WhC", "WeC", "ctxprojC",
                       "WfV", "WhV", "WeV", "ctxprojV"):
                t = cpool.tile(list(io[nm].shape), f16, tag=nm)
                nc.sync.dma_start(t[:], io[nm][:])
                wt[nm] = t

            for side, sd in (("A", "C"), ("B", "V")):
                pl = meta[side]
                nwin = pl["nwin"]
                dstall = cpool.tile([P, pl["T"]], f32, tag=f"dst{side}")
                nc.sync.dma_start(dstall[:], io[f"dstT{side}"][:])
                wall = cpool.tile([P, pl["T"]], f16, tag=f"w{side}")
                nc.sync.dma_start(wall[:], io[f"wT{side}"][:])

                off = 0
                tbase = 0
                hTpair = None
                for w in range(nwin):
                    slots = pl["npad"][w]
                    nt = slots // P
                    xw = xpool.tile([P, maxslot], f16, tag="xw")
                    nc.sync.dma_start(xw[:, :slots],
                                      io[f"xT{side}"][:, off:off + slots])
                    hps = psH.tile([P, WIN], f32, tag="hps")
                    for j0 in range(0, nt, GRP):
                        gn = min(GRP, nt - j0)
                        mps = psA.tile([P, GRP * P], f32, tag="mps")
                        msb = wpool.tile([P, GRP * P], f16, tag="msb")
                        ohws = []
                        for jj in range(gn):
                            t = tbase + j0 + jj
                            nc.tensor.matmul(mps[:, jj * P:(jj + 1) * P],
                                             lhsT=xw[:, (j0 + jj) * P:
                                                     (j0 + jj + 1) * P],
                                             rhs=wt[f"Wemb{side}"][:],
                                             start=True, stop=True)
                            # one-hot of dst scaled by 1/deg (0 on pad slots)
                            ohw = opool.tile([P, WIN], f16, tag="ohw")
                            nc.vector.scalar_tensor_tensor(
                                out=ohw[:], in0=iota16[:],
                                scalar=dstall[:, t:t + 1],
                                in1=wall[:, t:t + 1].to_broadcast([P, WIN]),
                                op0=mybir.AluOpType.is_equal,
                                op1=mybir.AluOpType.mult)
                            ohws.append(ohw)
                        nc.scalar.activation(
                            msb[:, :gn * P], mps[:, :gn * P],
                            mybir.ActivationFunctionType.Prelu, alpha=0.1)
                        for jj in range(gn):
                            j = j0 + jj
                            nc.tensor.matmul(hps[:],
                                             lhsT=msb[:, jj * P:(jj + 1) * P],
                                             rhs=ohws[jj][:],
                                             start=(j == 0), stop=(j == nt - 1),
                                             skip_group_check=True)
                    off += slots
                    tbase += nt

                    if hTpair is None:
                        hTpair = wpool.tile([P, 2 * WIN], f16, tag="hT")
                    nc.scalar.copy(
                        hTpair[:, (w % 2) * WIN:(w % 2) * WIN + WIN], hps[:])

                    if w % 2 == 0 and w != nwin - 1:
                        continue
                    # node MLP for the last 1-2 windows (N = 512 or 256)
                    nw = WIN if w % 2 == 0 else 2 * WIN
                    cga = (w + 1) * WIN - nw
                    featsl = wpool.tile([17, 2 * WIN], f16, tag="featsl")
                    nc.sync.dma_start(featsl[:, :nw],
                                      io[f"featsT{sd}"][:, cga:cga + nw])
                    embl = wpool.tile([P, 2 * WIN], f16, tag="embl")
                    nc.sync.dma_start(embl[:, :nw],
                                      io[f"embT{sd}"][:, cga:cga + nw])
                    ohul = wpool.tile([64, 2 * WIN], f16, tag="ohul")
                    nc.sync.dma_start(ohul[:, :nw],
                                      io[f"ohuT{sd}"][:, cga:cga + nw])
                    nps = psN.tile([P, 2 * WIN], f32, tag="nps")
                    nc.tensor.matmul(nps[:, :nw], lhsT=wt[f"Wf{sd}"][:],
                                     rhs=featsl[:, :nw], start=True, stop=False)
                    nc.tensor.matmul(nps[:, :nw], lhsT=wt[f"Wh{sd}"][:],
                                     rhs=hTpair[:, :nw],
                                     start=False, stop=False)
                    nc.tensor.matmul(nps[:, :nw], lhsT=wt[f"ctxproj{sd}"][:],
                                     rhs=ohul[:, :nw], start=False, stop=False)
                    nc.tensor.matmul(nps[:, :nw], lhsT=wt[f"We{sd}"][:],
                                     rhs=embl[:, :nw], start=False, stop=True)
                    nsb = wpool.tile([P, 2 * WIN], f16, tag="nsb")
                    nc.scalar.activation(nsb[:, :nw], nps[:, :nw],
                                         mybir.ActivationFunctionType.Prelu,
                                         alpha=0.1)
                    nc.sync.dma_start(io[f"out{sd}"][:, cga:cga + nw],
                                      nsb[:, :nw])
                    hTpair = None
    nc.compile()
    return nc


_spmd_state = {}

REPLICATED = ("WembA", "WembB",
              "WfC", "WhC", "WeC", "ctxprojC",
              "WfV", "WhV", "WeV", "ctxprojV")


def _run_spmd(nc, stacked_map, repl_map):
    """One shard_map dispatch running the identical program on all 8 cores.

    stacked_map[nm] is the core-major stacked array [M*rows, ...]; repl_map
    holds the small replicated weights."""
    import concourse.mybir as mybir
    import jax
    from concourse.bass2jax import (_bass_exec_p, install_neuronx_cc_hook,
                                    partition_id_tensor)
    from jax.experimental.shard_map import shard_map
    from jax.sharding import Mesh, NamedSharding, PartitionSpec

    install_neuronx_cc_hook()
    partition_name = nc.partition_id_tensor.name if nc.partition_id_tensor else None
    in_names, out_names, out_avals, zero_shapes = [], [], [], []
    for alloc in nc.m.functions[0].allocations:
        if not isinstance(alloc, mybir.MemoryLocationSet):
            continue
        name = alloc.memorylocations[0].name
        if alloc.kind == "ExternalInput":
            if name != partition_name:
                in_names.append(name)
        elif alloc.kind == "ExternalOutput":
            shape = tuple(alloc.tensor_shape)
            dtype = mybir.dt.np(alloc.dtype)
            out_names.append(name)
            out_avals.append(jax.core.ShapedArray(shape, dtype))
            zero_shapes.append((shape, dtype))
    n_params = len(in_names)
    n_outs = len(out_names)
    all_names = list(in_names) + list(out_names)
    if partition_name is not None:
        all_names.append(partition_name)
    donate = tuple(range(n_params, n_params + n_outs))

    def _body(*args):
        operands = list(args)
        if partition_name is not None:
            operands.append(partition_id_tensor())
        return tuple(_bass_exec_p.bind(
            *operands, out_avals=tuple(out_avals), in_names=tuple(all_names),
            out_names=tuple(out_names), lowering_input_output_aliases=(),
            sim_require_finite=True, sim_require_nnan=True, nc=nc))

    devices = jax.devices()[:M]
    mesh = Mesh(np.asarray(devices), ("core",))
    in_specs = tuple(
        PartitionSpec() if nm in REPLICATED else PartitionSpec("core")
        for nm in in_names) + (PartitionSpec("core"),) * n_outs
    out_specs = (PartitionSpec("core"),) * n_outs
    sharded = jax.jit(
        shard_map(_body, mesh=mesh, in_specs=in_specs, out_specs=out_specs,
                  check_rep=False),
        donate_argnums=donate, keep_unused=True)

    sh_core = NamedSharding(mesh, PartitionSpec("core"))
    sh_repl = NamedSharding(mesh, PartitionSpec())

    d_ins = []
    for nm in in_names:
        if nm in REPLICATED:
            d_ins.append(jax.device_put(repl_map[nm], sh_repl))
        else:
            d_ins.append(jax.device_put(stacked_map[nm], sh_core))
    d_zeros = [jax.device_put(np.zeros((M * s[0], *s[1:]), dt), sh_core)
               for (s, dt) in zero_shapes]

    outs = sharded(*d_ins, *d_zeros)
    host = {nm: np.asarray(o) for nm, o in zip(out_names, outs)}

    _spmd_state.clear()
    _spmd_state.update(fn=sharded, d_ins=d_ins, outs=list(outs),
                       out_names=out_names, sh_core=sh_core,
                       zero_shapes=zero_shapes, nc=nc)
    return host


def _redispatch():
    """Re-run the compiled program once (fresh zero output buffers)."""
    import jax
    st = _spmd_state
    d_zeros = [jax.device_put(np.zeros((M * s[0], *s[1:]), dt), st["sh_core"])
               for (s, dt) in st["zero_shapes"]]
    outs = st["fn"](*st["d_ins"], *d_zeros)
    st["outs"] = list(outs)
    return {nm: np.asarray(o) for nm, o in zip(st["out_names"], outs)}


def _assemble(host_out, n_nodes, Np):
    """[M*P, Np] core-major d-major output -> [n_nodes, 128] f32."""
    nn = n_nodes // M
    parts = [host_out[k * P:(k + 1) * P, :nn] for k in range(M)]
    return np.concatenate(parts, axis=1).T.astype(F32)


def _segmean(x, ids, n):
    order = np.argsort(ids, kind="stable")
    xs = x[order]
    ids_s = ids[order]
    starts = np.searchsorted(ids_s, np.arange(n))
    cnt = np.bincount(ids, minlength=n).astype(F32)
    sums = np.zeros((n, x.shape[1]), F32)
    nz = cnt > 0
    if nz.any():
        sums[nz] = np.add.reduceat(xs, starts[nz], axis=0)
    return sums / np.maximum(cnt, 1.0)[:, None]


def _spot_check(new_clause, new_var, inp, a_src, a_dst, c_src, c_dst,
                clause_ctx, var_ctx, n_sample=48, seed=7):
    """Exact host recomputation of a few output nodes.  Clean device runs
    differ by <2e-2 absmax (f16 + sat-fold path); silent corruption differs
    by ~2.5 — threshold 0.2 separates them cleanly."""
    rng = np.random.default_rng(seed)
    lrelu = lambda x: np.where(x >= 0, x, 0.1 * x)
    ok = True
    for (emb_src, src, dst, sat, Wm, bm, feats, Wn, bn, ctx_ids, emb_self,
         got_all) in (
            (inp["var_emb"], a_src, a_dst, inp["edge_sat_vc"], inp["W_vc"],
             inp["b_vc"], inp["clause_feats"], inp["W_c"], inp["b_c"],
             clause_ctx, inp["clause_emb"], new_clause),
            (inp["clause_emb"], c_src, c_dst, inp["edge_sat_cv"], inp["W_cv"],
             inp["b_cv"], inp["var_feats"], inp["W_v"], inp["b_v"],
             var_ctx, inp["var_emb"], new_var)):
        n_nodes = feats.shape[0]
        sample = rng.choice(n_nodes, size=min(n_sample, n_nodes), replace=False)
        sel = np.isin(dst, sample)
        es, ds = src[sel], dst[sel]
        m = lrelu(np.concatenate([sat[sel].astype(F32),
                                  emb_src[es].astype(F32)], 1)
                  @ Wm.astype(F32) + bm.astype(F32))
        h = np.zeros((n_nodes, 128), F32)
        np.add.at(h, ds, m)
        cnt = np.bincount(dst, minlength=n_nodes).astype(F32)
        h = h / np.maximum(cnt, 1.0)[:, None]
        ctx_e = inp["ctx_emb"][ctx_ids[sample]].astype(F32)
        z = np.concatenate([feats[sample].astype(F32), h[sample], ctx_e,
                            emb_self[sample].astype(F32)], 1) \
            @ Wn.astype(F32) + bn.astype(F32)
        ref = lrelu(z)
        if np.abs(got_all[sample] - ref).max() > 0.2:
            ok = False
    return ok


def kernel(**inputs):
    inp = {k: np.asarray(v) for k, v in inputs.items()}
    var_emb, clause_emb, ctx_emb = inp["var_emb"], inp["clause_emb"], inp["ctx_emb"]
    nv, ncl, nu = var_emb.shape[0], clause_emb.shape[0], ctx_emb.shape[0]

    a_src = inp["assigns_src"].astype(np.int64)
    a_dst = inp["assigns_dst"].astype(np.int64)
    c_src = inp["contains_src"].astype(np.int64)
    c_dst = inp["contains_dst"].astype(np.int64)
    var_ctx = inp["var_ctx"].astype(np.int64)
    clause_ctx = inp["clause_ctx"].astype(np.int64)

    # fold [sat, 1] @ [W_sat; b] through W_emb^{-1} into the gathered x rows
    W_vc, b_vc = inp["W_vc"].astype(F32), inp["b_vc"].astype(F32)
    W_cv, b_cv = inp["W_cv"].astype(F32), inp["b_cv"].astype(F32)
    WsbInvA = (np.vstack([W_vc[:4], b_vc[None, :]]).astype(np.float64)
               @ np.linalg.inv(W_vc[4:132].astype(np.float64))).astype(F32)
    WsbInvB = (np.vstack([W_cv[:4], b_cv[None, :]]).astype(np.float64)
               @ np.linalg.inv(W_cv[4:132].astype(np.float64))).astype(F32)

    planA = _side_prep(a_src, a_dst, inp["edge_sat_vc"],
                       var_emb.astype(F32), WsbInvA, ncl)
    planB = _side_prep(c_src, c_dst, inp["edge_sat_cv"],
                       clause_emb.astype(F32), WsbInvB, nv)

    NpC = planA["nwin"] * WIN
    NpV = planB["nwin"] * WIN
    emb16V = var_emb.astype(F16)
    emb16C = clause_emb.astype(F16)
    featsTC, embTC, ohuTC = _node_prep(inp["clause_feats"], emb16C,
                                       clause_ctx, ncl, NpC)
    featsTV, embTV, ohuTV = _node_prep(inp["var_feats"], emb16V,
                                       var_ctx, nv, NpV)

    def node_w(Wn, bn):
        Wn, bn = Wn.astype(F32), bn.astype(F32)
        nf = Wn.shape[0] - 3 * 128
        Wf = np.vstack([Wn[:nf], bn[None, :]]).astype(F16)
        Wh = np.ascontiguousarray(Wn[nf:nf + 128]).astype(F16)
        ctxproj = (ctx_emb.astype(F32) @ Wn[nf + 128:nf + 256]).astype(F16)
        We = np.ascontiguousarray(Wn[nf + 256:nf + 384]).astype(F16)
        return Wf, Wh, ctxproj, We

    WfC, WhC, ctxprojC, WeC = node_w(inp["W_c"], inp["b_c"])
    WfV, WhV, ctxprojV, WeV = node_w(inp["W_v"], inp["b_v"])

    repl_map = dict(
        WembA=np.ascontiguousarray(W_vc[4:132]).astype(F16),
        WembB=np.ascontiguousarray(W_cv[4:132]).astype(F16),
        WfC=WfC, WhC=WhC, WeC=WeC, ctxprojC=ctxprojC,
        WfV=WfV, WhV=WhV, WeV=WeV, ctxprojV=ctxprojV,
    )
    stacked_map = dict(
        xTA=planA["xT"], dstTA=planA["dstT"], wTA=planA["wT"],
        xTB=planB["xT"], dstTB=planB["dstT"], wTB=planB["wT"],
        featsTC=featsTC, embTC=embTC, ohuTC=ohuTC,
        featsTV=featsTV, embTV=embTV, ohuTV=ohuTV,
    )

    meta = dict(
        A=dict(nwin=planA["nwin"], npad=planA["npad"], S=planA["S"], T=planA["T"]),
        B=dict(nwin=planB["nwin"], npad=planB["npad"], S=planB["S"], T=planB["T"]),
        NpC=NpC, NpV=NpV)
    nc = _build_program(meta)
    host = _run_spmd(nc, stacked_map, repl_map)

    new_clause = _assemble(host["outC"], ncl, NpC)
    new_var = _assemble(host["outV"], nv, NpV)
    # guard against rare silent corruption on the terminal
    for _ in range(2):
        if _spot_check(new_clause, new_var, inp, a_src, a_dst, c_src, c_dst,
                       clause_ctx, var_ctx):
            break
        host = _redispatch()
        new_clause = _assemble(host["outC"], ncl, NpC)
        new_var = _assemble(host["outV"], nv, NpV)

    # Phase 3 on host
    c_ctx = _segmean(new_clause, clause_ctx, nu)
    v_ctx = _segmean(new_var, var_ctx, nu)
    zu = np.concatenate([inp["ctx_feats"].astype(F32), c_ctx, v_ctx,
                         ctx_emb.astype(F32)], 1) @ inp["W_u"].astype(F32) \
        + inp["b_u"].astype(F32)
    new_ctx = np.where(zu >= 0, zu, 0.1 * zu).astype(F32)

    return np.concatenate([new_clause, new_var, new_ctx], 0).astype(F32)


# revision 17
# speedup vs baseline: 1.1966x; 1.1966x over previous
"""Trainium2 Bass kernel for nn_MessageGNN (gnn_message_passing).

Sharding: destination-sharded edges across 8 cores.  Core k owns clauses
[k*50000,(k+1)*50000) and vars [k*12500,(k+1)*12500) plus every edge whose
destination falls in its slice, so segment sums are fully core-local.
All 8 cores run ONE identical Bass program (SPMD); per-core variation
lives entirely in the data.  The edge schedule is made uniform by padding
every 256-destination window's edge count to the max over the 8 cores
(pad slots carry zero x, weight 0, dst sentinel 70000).

Host-side preprocessing folds everything per-edge into one pre-gathered
feature-major payload: x'_e = x_src(e) + ([sat_e, 1] @ [W_sat; b] @
W_emb^{-1}), so the edge MLP is a single f16 matmul z = x' W_emb per
128-edge tile (W_emb is square and well-conditioned; the fold costs
~6e-3 absolute at the edge stage, well under the 2e-2 gate).  A DVE
one-hot (iota==dst)*1/deg [e,256] and one N=256 matmul accumulate each
window's h^T [128d, 256dst] in PSUM; Prelu runs grouped on the Scalar
engine.  The node MLP consumes two windows at a time (N=512): 4 matmuls
(feats+bias / h / host-projected ctx one-hot / emb), outputs leave
d-major [128, Np] and the host transposes.  Phase 3 (context update)
runs on host from the returned node embeddings.
"""

import sys

sys.path.insert(0, "/opt/trn_rl_repo")

import numpy as np

M = 8
WIN = 256
P = 128
GRP = 4
PAD_DST = 70000.0

F16 = np.float16
F32 = np.float32


def _side_prep(src, dst, sat, emb32, WsbInv, n_dst):
    """Sort edges by destination, build the shared padded window schedule
    and the per-core slot arrays (stacked core-major for shard_map).

    x' = emb[src] + [sat, 1] @ WsbInv  (sat/bias folded into emb space)."""
    ndc = n_dst // M
    nwin = -(-ndc // WIN)
    order = np.argsort(dst, kind="stable")
    src_s, dst_s, sat_s = src[order], dst[order], sat[order]
    bounds = np.searchsorted(dst_s, np.arange(M + 1) * ndc)

    cnts = np.zeros((M, nwin), np.int64)
    wlocs = []
    for k in range(M):
        lo, hi = bounds[k], bounds[k + 1]
        wloc = (dst_s[lo:hi] - k * ndc) // WIN
        cnts[k] = np.bincount(wloc, minlength=nwin)
        wlocs.append(wloc)
    npad = np.maximum(((cnts.max(0) + P - 1) // P) * P, P)  # [nwin]
    offs = np.zeros(nwin + 1, np.int64)
    offs[1:] = np.cumsum(npad)
    S = int(offs[-1])
    T = S // P

    wrec = (1.0 / np.maximum(np.bincount(dst, minlength=n_dst), 1.0)).astype(F32)

    xT = np.zeros((M * P, S), F16)
    dstw = np.full((M, S), PAD_DST, F32)
    wsc = np.zeros((M, S), F32)
    for k in range(M):
        lo, hi = bounds[k], bounds[k + 1]
        wloc = wlocs[k]
        runstart = np.zeros(nwin, np.int64)
        runstart[1:] = np.cumsum(cnts[k][:-1])
        n = hi - lo
        pos = offs[wloc] + (np.arange(n) - runstart[wloc])
        xp = emb32[src_s[lo:hi]] + sat_s[lo:hi].astype(F32) @ WsbInv[:4] \
            + WsbInv[4]
        xT[k * P:(k + 1) * P, pos] = xp.astype(F16).T
        dstw[k, pos] = (dst_s[lo:hi] - k * ndc - wloc * WIN).astype(F32)
        wsc[k, pos] = wrec[dst_s[lo:hi]]
    dstT = np.ascontiguousarray(
        dstw.reshape(M, T, P).transpose(0, 2, 1).reshape(M * P, T))
    wT = np.ascontiguousarray(
        wsc.reshape(M, T, P).transpose(0, 2, 1).reshape(M * P, T)).astype(F16)
    return dict(nwin=nwin, npad=npad.tolist(), S=S, T=T,
                xT=xT, dstT=dstT, wT=wT)


def _node_prep(feats, emb16, ctx_ids, n_nodes, Np):
    """Per-core node-phase arrays, stacked core-major."""
    nn = n_nodes // M
    nf = feats.shape[1]
    featsT = np.zeros((M * (nf + 1), Np), F16)
    embT = np.zeros((M * P, Np), F16)
    ohuT = np.zeros((M * 64, Np), F16)
    for k in range(M):
        fs, es, cs_ = (a[k * nn:(k + 1) * nn] for a in (feats, emb16, ctx_ids))
        featsT[k * (nf + 1):k * (nf + 1) + nf, :nn] = fs.T.astype(F16)
        featsT[k * (nf + 1) + nf, :nn] = 1.0
        embT[k * P:(k + 1) * P, :nn] = es.T
        ohuT[k * 64 + cs_, np.arange(nn)] = 1.0
    return featsT, embT, ohuT


def _build_program(meta):
    import concourse.mybir as mybir
    import concourse.tile as tile
    from concourse import bacc

    f16, f32 = mybir.dt.float16, mybir.dt.float32

    nc = bacc.Bacc("TRN2", target_bir_lowering=False, debug=False, num_devices=1)
    io = {}

    def dram(name, shape, dt, kind="ExternalInput"):
        io[name] = nc.dram_tensor(name, list(shape), dt, kind=kind)
        return io[name]

    for side in ("A", "B"):
        pl = meta[side]
        dram(f"xT{side}", [P, pl["S"]], f16)
        dram(f"dstT{side}", [P, pl["T"]], f32)
        dram(f"wT{side}", [P, pl["T"]], f16)
        dram(f"Wemb{side}", [P, P], f16)
    for sd in ("C", "V"):
        Np = meta[f"Np{sd}"]
        dram(f"featsT{sd}", [17, Np], f16)
        dram(f"embT{sd}", [P, Np], f16)
        dram(f"ohuT{sd}", [64, Np], f16)
        dram(f"Wf{sd}", [17, P], f16)
        dram(f"Wh{sd}", [P, P], f16)
        dram(f"We{sd}", [P, P], f16)
        dram(f"ctxproj{sd}", [64, P], f16)
        dram(f"out{sd}", [P, Np], f16, kind="ExternalOutput")

    maxslot = max(max(meta["A"]["npad"]), max(meta["B"]["npad"]))

    with tile.TileContext(nc) as tc:
        with tc.tile_pool(name="const", bufs=1) as cpool, \
             tc.tile_pool(name="xs", bufs=3) as xpool, \
             tc.tile_pool(name="work", bufs=5) as wpool, \
             tc.tile_pool(name="oh", bufs=12) as opool, \
             tc.tile_pool(name="psA", bufs=2, space="PSUM") as psA, \
             tc.tile_pool(name="psH", bufs=2, space="PSUM") as psH, \
             tc.tile_pool(name="psN", bufs=2, space="PSUM") as psN:

            iota_i = cpool.tile([P, WIN], mybir.dt.int32)
            nc.gpsimd.iota(iota_i[:], pattern=[[1, WIN]], base=0,
                           channel_multiplier=0)
            iota16 = cpool.tile([P, WIN], f16)
            nc.vector.tensor_copy(iota16[:], iota_i[:])

            wt = {}
            for nm in ("WembA", "WembB",
                       "WfC", "WhC", "WeC", "ctxprojC",
                       "WfV", "WhV", "WeV", "ctxprojV"):
                t = cpool.tile(list(io[nm].shape), f16, tag=nm)
                nc.sync.dma_start(t[:], io[nm][:])
                wt[nm] = t

            for side, sd in (("A", "C"), ("B", "V")):
                pl = meta[side]
                nwin = pl["nwin"]
                dstall = cpool.tile([P, pl["T"]], f32, tag=f"dst{side}")
                nc.sync.dma_start(dstall[:], io[f"dstT{side}"][:])
                wall = cpool.tile([P, pl["T"]], f16, tag=f"w{side}")
                nc.sync.dma_start(wall[:], io[f"wT{side}"][:])

                off = 0
                tbase = 0
                hTpair = None
                for w in range(nwin):
                    slots = pl["npad"][w]
                    nt = slots // P
                    xw = xpool.tile([P, maxslot], f16, tag="xw")
                    nc.sync.dma_start(xw[:, :slots],
                                      io[f"xT{side}"][:, off:off + slots])
                    hps = psH.tile([P, WIN], f32, tag="hps")
                    for j0 in range(0, nt, GRP):
                        gn = min(GRP, nt - j0)
                        mps = psA.tile([P, GRP * P], f32, tag="mps")
                        msb = wpool.tile([P, GRP * P], f16, tag="msb")
                        ohws = []
                        for jj in range(gn):
                            t = tbase + j0 + jj
                            nc.tensor.matmul(mps[:, jj * P:(jj + 1) * P],
                                             lhsT=xw[:, (j0 + jj) * P:
                                                     (j0 + jj + 1) * P],
                                             rhs=wt[f"Wemb{side}"][:],
                                             start=True, stop=True)
                            # one-hot of dst scaled by 1/deg (0 on pad slots)
                            ohw = opool.tile([P, WIN], f16, tag="ohw")
                            nc.vector.scalar_tensor_tensor(
                                out=ohw[:], in0=iota16[:],
                                scalar=dstall[:, t:t + 1],
                                in1=wall[:, t:t + 1].to_broadcast([P, WIN]),
                                op0=mybir.AluOpType.is_equal,
                                op1=mybir.AluOpType.mult)
                            ohws.append(ohw)
                        nc.scalar.activation(
                            msb[:, :gn * P], mps[:, :gn * P],
                            mybir.ActivationFunctionType.Prelu, alpha=0.1)
                        for jj in range(gn):
                            j = j0 + jj
                            nc.tensor.matmul(hps[:],
                                             lhsT=msb[:, jj * P:(jj + 1) * P],
                                             rhs=ohws[jj][:],
                                             start=(j == 0), stop=(j == nt - 1),
                                             skip_group_check=True)
                    off += slots
                    tbase += nt

                    if hTpair is None:
                        hTpair = wpool.tile([P, 2 * WIN], f16, tag="hT")
                    nc.scalar.copy(
                        hTpair[:, (w % 2) * WIN:(w % 2) * WIN + WIN], hps[:])

                    if w % 2 == 0 and w != nwin - 1:
                        continue
                    # node MLP for the last 1-2 windows (N = 512 or 256)
                    nw = WIN if w % 2 == 0 else 2 * WIN
                    cga = (w + 1) * WIN - nw
                    featsl = wpool.tile([17, 2 * WIN], f16, tag="featsl")
                    nc.sync.dma_start(featsl[:, :nw],
                                      io[f"featsT{sd}"][:, cga:cga + nw])
                    embl = wpool.tile([P, 2 * WIN], f16, tag="embl")
                    nc.sync.dma_start(embl[:, :nw],
                                      io[f"embT{sd}"][:, cga:cga + nw])
                    ohul = wpool.tile([64, 2 * WIN], f16, tag="ohul")
                    nc.sync.dma_start(ohul[:, :nw],
                                      io[f"ohuT{sd}"][:, cga:cga + nw])
                    nps = psN.tile([P, 2 * WIN], f32, tag="nps")
                    nc.tensor.matmul(nps[:, :nw], lhsT=wt[f"Wf{sd}"][:],
                                     rhs=featsl[:, :nw], start=True, stop=False)
                    nc.tensor.matmul(nps[:, :nw], lhsT=wt[f"Wh{sd}"][:],
                                     rhs=hTpair[:, :nw],
                                     start=False, stop=False)
                    nc.tensor.matmul(nps[:, :nw], lhsT=wt[f"ctxproj{sd}"][:],
                                     rhs=ohul[:, :nw], start=False, stop=False)
                    nc.tensor.matmul(nps[:, :nw], lhsT=wt[f"We{sd}"][:],
                                     rhs=embl[:, :nw], start=False, stop=True)
                    nsb = wpool.tile([P, 2 * WIN], f16, tag="nsb")
                    nc.scalar.activation(nsb[:, :nw], nps[:, :nw],
                                         mybir.ActivationFunctionType.Prelu,
                                         alpha=0.1)
                    nc.sync.dma_start(io[f"out{sd}"][:, cga:cga + nw],
                                      nsb[:, :nw])
                    hTpair = None
    nc.compile()
    return nc


_spmd_state = {}

REPLICATED = ("WembA", "WembB",
              "WfC", "WhC", "WeC", "ctxprojC",
              "WfV", "WhV", "WeV", "ctxprojV")


def _run_spmd(nc, stacked_map, repl_map):
    """One shard_map dispatch running the identical program on all 8 cores.

    stacked_map[nm] is the core-major stacked array [M*rows, ...]; repl_map
    holds the small replicated weights."""
    import concourse.mybir as mybir
    import jax
    from concourse.bass2jax import (_bass_exec_p, install_neuronx_cc_hook,
                                    partition_id_tensor)
    from jax.experimental.shard_map import shard_map
    from jax.sharding import Mesh, NamedSharding, PartitionSpec

    install_neuronx_cc_hook()
    partition_name = nc.partition_id_tensor.name if nc.partition_id_tensor else None
    in_names, out_names, out_avals, zero_shapes = [], [], [], []
    for alloc in nc.m.functions[0].allocations:
        if not isinstance(alloc, mybir.MemoryLocationSet):
            continue
        name = alloc.memorylocations[0].name
        if alloc.kind == "ExternalInput":
            if name != partition_name:
                in_names.append(name)
        elif alloc.kind == "ExternalOutput":
            shape = tuple(alloc.tensor_shape)
            dtype = mybir.dt.np(alloc.dtype)
            out_names.append(name)
            out_avals.append(jax.core.ShapedArray(shape, dtype))
            zero_shapes.append((shape, dtype))
    n_params = len(in_names)
    n_outs = len(out_names)
    all_names = list(in_names) + list(out_names)
    if partition_name is not None:
        all_names.append(partition_name)
    donate = tuple(range(n_params, n_params + n_outs))

    def _body(*args):
        operands = list(args)
        if partition_name is not None:
            operands.append(partition_id_tensor())
        return tuple(_bass_exec_p.bind(
            *operands, out_avals=tuple(out_avals), in_names=tuple(all_names),
            out_names=tuple(out_names), lowering_input_output_aliases=(),
            sim_require_finite=True, sim_require_nnan=True, nc=nc))

    devices = jax.devices()[:M]
    mesh = Mesh(np.asarray(devices), ("core",))
    in_specs = tuple(
        PartitionSpec() if nm in REPLICATED else PartitionSpec("core")
        for nm in in_names) + (PartitionSpec("core"),) * n_outs
    out_specs = (PartitionSpec("core"),) * n_outs
    sharded = jax.jit(
        shard_map(_body, mesh=mesh, in_specs=in_specs, out_specs=out_specs,
                  check_rep=False),
        donate_argnums=donate, keep_unused=True)

    sh_core = NamedSharding(mesh, PartitionSpec("core"))
    sh_repl = NamedSharding(mesh, PartitionSpec())

    d_ins = []
    for nm in in_names:
        if nm in REPLICATED:
            d_ins.append(jax.device_put(repl_map[nm], sh_repl))
        else:
            d_ins.append(jax.device_put(stacked_map[nm], sh_core))
    d_zeros = [jax.device_put(np.zeros((M * s[0], *s[1:]), dt), sh_core)
               for (s, dt) in zero_shapes]

    outs = sharded(*d_ins, *d_zeros)
    host = {nm: np.asarray(o) for nm, o in zip(out_names, outs)}

    _spmd_state.clear()
    _spmd_state.update(fn=sharded, d_ins=d_ins, outs=list(outs),
                       out_names=out_names, sh_core=sh_core,
                       zero_shapes=zero_shapes, nc=nc)
    return host


def _redispatch():
    """Re-run the compiled program once (fresh zero output buffers)."""
    import jax
    st = _spmd_state
    d_zeros = [jax.device_put(np.zeros((M * s[0], *s[1:]), dt), st["sh_core"])
               for (s, dt) in st["zero_shapes"]]
    outs = st["fn"](*st["d_ins"], *d_zeros)
    st["outs"] = list(outs)
    return {nm: np.asarray(o) for nm, o in zip(st["out_names"], outs)}


def _assemble(host_out, n_nodes, Np):
    """[M*P, Np] core-major d-major output -> [n_nodes, 128] f32."""
    nn = n_nodes // M
    parts = [host_out[k * P:(k + 1) * P, :nn] for k in range(M)]
    return np.concatenate(parts, axis=1).T.astype(F32)


def _segmean(x, ids, n):
    order = np.argsort(ids, kind="stable")
    xs = x[order]
    ids_s = ids[order]
    starts = np.searchsorted(ids_s, np.arange(n))
    cnt = np.bincount(ids, minlength=n).astype(F32)
    sums = np.zeros((n, x.shape[1]), F32)
    nz = cnt > 0
    if nz.any():
        sums[nz] = np.add.reduceat(xs, starts[nz], axis=0)
    return sums / np.maximum(cnt, 1.0)[:, None]


def _spot_check(new_clause, new_var, inp, a_src, a_dst, c_src, c_dst,
                clause_ctx, var_ctx, n_sample=48, seed=7):
    """Exact host recomputation of a few output nodes.  Clean device runs
    differ by <2e-2 absmax (f16 + sat-fold path); silent corruption differs
    by ~2.5 — threshold 0.2 separates them cleanly."""
    rng = np.random.default_rng(seed)
    lrelu = lambda x: np.where(x >= 0, x, 0.1 * x)
    ok = True
    for (emb_src, src, dst, sat, Wm, bm, feats, Wn, bn, ctx_ids, emb_self,
         got_all) in (
            (inp["var_emb"], a_src, a_dst, inp["edge_sat_vc"], inp["W_vc"],
             inp["b_vc"], inp["clause_feats"], inp["W_c"], inp["b_c"],
             clause_ctx, inp["clause_emb"], new_clause),
            (inp["clause_emb"], c_src, c_dst, inp["edge_sat_cv"], inp["W_cv"],
             inp["b_cv"], inp["var_feats"], inp["W_v"], inp["b_v"],
             var_ctx, inp["var_emb"], new_var)):
        n_nodes = feats.shape[0]
        sample = rng.choice(n_nodes, size=min(n_sample, n_nodes), replace=False)
        sel = np.isin(dst, sample)
        es, ds = src[sel], dst[sel]
        m = lrelu(np.concatenate([sat[sel].astype(F32),
                                  emb_src[es].astype(F32)], 1)
                  @ Wm.astype(F32) + bm.astype(F32))
        h = np.zeros((n_nodes, 128), F32)
        np.add.at(h, ds, m)
        cnt = np.bincount(dst, minlength=n_nodes).astype(F32)
        h = h / np.maximum(cnt, 1.0)[:, None]
        ctx_e = inp["ctx_emb"][ctx_ids[sample]].astype(F32)
        z = np.concatenate([feats[sample].astype(F32), h[sample], ctx_e,
                            emb_self[sample].astype(F32)], 1) \
            @ Wn.astype(F32) + bn.astype(F32)
        ref = lrelu(z)
        if np.abs(got_all[sample] - ref).max() > 0.2:
            ok = False
    return ok


def kernel(**inputs):
    inp = {k: np.asarray(v) for k, v in inputs.items()}
    var_emb, clause_emb, ctx_emb = inp["var_emb"], inp["clause_emb"], inp["ctx_emb"]
    nv, ncl, nu = var_emb.shape[0], clause_emb.shape[0], ctx_emb.shape[0]

    a_src = inp["assigns_src"].astype(np.int64)
    a_dst = inp["assigns_dst"].astype(np.int64)
    c_src = inp["contains_src"].astype(np.int64)
    c_dst = inp["contains_dst"].astype(np.int64)
    var_ctx = inp["var_ctx"].astype(np.int64)
    clause_ctx = inp["clause_ctx"].astype(np.int64)

    # fold [sat, 1] @ [W_sat; b] through W_emb^{-1} into the gathered x rows
    W_vc, b_vc = inp["W_vc"].astype(F32), inp["b_vc"].astype(F32)
    W_cv, b_cv = inp["W_cv"].astype(F32), inp["b_cv"].astype(F32)
    WsbInvA = (np.vstack([W_vc[:4], b_vc[None, :]]).astype(np.float64)
               @ np.linalg.inv(W_vc[4:132].astype(np.float64))).astype(F32)
    WsbInvB = (np.vstack([W_cv[:4], b_cv[None, :]]).astype(np.float64)
               @ np.linalg.inv(W_cv[4:132].astype(np.float64))).astype(F32)

    planA = _side_prep(a_src, a_dst, inp["edge_sat_vc"],
                       var_emb.astype(F32), WsbInvA, ncl)
    planB = _side_prep(c_src, c_dst, inp["edge_sat_cv"],
                       clause_emb.astype(F32), WsbInvB, nv)

    NpC = planA["nwin"] * WIN
    NpV = planB["nwin"] * WIN
    emb16V = var_emb.astype(F16)
    emb16C = clause_emb.astype(F16)
    featsTC, embTC, ohuTC = _node_prep(inp["clause_feats"], emb16C,
                                       clause_ctx, ncl, NpC)
    featsTV, embTV, ohuTV = _node_prep(inp["var_feats"], emb16V,
                                       var_ctx, nv, NpV)

    def node_w(Wn, bn):
        Wn, bn = Wn.astype(F32), bn.astype(F32)
        nf = Wn.shape[0] - 3 * 128
        Wf = np.vstack([Wn[:nf], bn[None, :]]).astype(F16)
        Wh = np.ascontiguousarray(Wn[nf:nf + 128]).astype(F16)
        ctxproj = (ctx_emb.astype(F32) @ Wn[nf + 128:nf + 256]).astype(F16)
        We = np.ascontiguousarray(Wn[nf + 256:nf + 384]).astype(F16)
        return Wf, Wh, ctxproj, We

    WfC, WhC, ctxprojC, WeC = node_w(inp["W_c"], inp["b_c"])
    WfV, WhV, ctxprojV, WeV = node_w(inp["W_v"], inp["b_v"])

    repl_map = dict(
        WembA=np.ascontiguousarray(W_vc[4:132]).astype(F16),
        WembB=np.ascontiguousarray(W_cv[4:132]).astype(F16),
        WfC=WfC, WhC=WhC, WeC=WeC, ctxprojC=ctxprojC,
        WfV=WfV, WhV=WhV, WeV=WeV, ctxprojV=ctxprojV,
    )
    stacked_map = dict(
        xTA=planA["xT"], dstTA=planA["dstT"], wTA=planA["wT"],
        xTB=planB["xT"], dstTB=planB["dstT"], wTB=planB["wT"],
        featsTC=featsTC, embTC=embTC, ohuTC=ohuTC,
        featsTV=featsTV, embTV=embTV, ohuTV=ohuTV,
    )

    meta = dict(
        A=dict(nwin=planA["nwin"], npad=planA["npad"], S=planA["S"], T=planA["T"]),
        B=dict(nwin=planB["nwin"], npad=planB["npad"], S=planB["S"], T=planB["T"]),
        NpC=NpC, NpV=NpV)
    nc = _build_program(meta)
    host = _run_spmd(nc, stacked_map, repl_map)

    new_clause = _assemble(host["outC"], ncl, NpC)
    new_var = _assemble(host["outV"], nv, NpV)
    # guard against rare silent corruption on the terminal
    for _ in range(2):
        if _spot_check(new_clause, new_var, inp, a_src, a_dst, c_src, c_dst,
                       clause_ctx, var_ctx):
            break
        host = _redispatch()
        new_clause = _assemble(host["outC"], ncl, NpC)
        new_var = _assemble(host["outV"], nv, NpV)

    # Phase 3 on host
    c_ctx = _segmean(new_clause, clause_ctx, nu)
    v_ctx = _segmean(new_var, var_ctx, nu)
    zu = np.concatenate([inp["ctx_feats"].astype(F32), c_ctx, v_ctx,
                         ctx_emb.astype(F32)], 1) @ inp["W_u"].astype(F32) \
        + inp["b_u"].astype(F32)
    new_ctx = np.where(zu >= 0, zu, 0.1 * zu).astype(F32)

    return np.concatenate([new_clause, new_var, new_ctx], 0).astype(F32)


# revision 23
# speedup vs baseline: 1.5598x; 1.3035x over previous
"""Trainium2 Bass kernel for nn_MessageGNN (gnn_message_passing).

Sharding: destination-sharded edges across 8 cores.  Core k owns clauses
[k*50000,(k+1)*50000) and vars [k*12500,(k+1)*12500) plus every edge whose
destination falls in its slice, so segment sums are fully core-local.
All 8 cores run ONE identical Bass program (SPMD); per-core variation
lives entirely in the data.  The edge schedule is made uniform by padding
every 256-destination window's edge count to the max over the 8 cores
(pad slots carry zero x, weight 0, dst sentinel 70000).

Host-side preprocessing folds everything per-edge into one pre-gathered
feature-major payload: x'_e = x_src(e) + ([sat_e, 1] @ [W_sat; b] @
W_emb^{-1}), so the edge MLP is a single f16 matmul z = x' W_emb per
128-edge tile (W_emb is square and well-conditioned; the fold costs
~6e-3 absolute at the edge stage, well under the 2e-2 gate).  A DVE
one-hot (iota==dst)*1/deg [e,256] and one N=256 matmul accumulate each
window's h^T [128d, 256dst] in PSUM; Prelu runs grouped on the Scalar
engine.  The node MLP consumes two windows at a time (N=512): 4 matmuls
(feats+bias / h / host-projected ctx one-hot / emb), outputs leave
d-major [128, Np] and the host transposes.  Phase 3 (context update)
runs on host from the returned node embeddings.
"""

import sys

sys.path.insert(0, "/opt/trn_rl_repo")

import numpy as np

M = 8
WIN = 256
P = 128
GRP = 4
PAD_DST = 70000.0

F16 = np.float16
F32 = np.float32


def _side_prep(src, dst, sat, emb32, WsbInv, n_dst):
    """Sort edges by destination, build the shared padded window schedule
    and the per-core slot arrays (stacked core-major for shard_map).

    x' = emb[src] + [sat, 1] @ WsbInv  (sat/bias folded into emb space)."""
    ndc = n_dst // M
    nwin = -(-ndc // WIN)
    order = np.argsort(dst, kind="stable")
    src_s, dst_s, sat_s = src[order], dst[order], sat[order]
    bounds = np.searchsorted(dst_s, np.arange(M + 1) * ndc)

    cnts = np.zeros((M, nwin), np.int64)
    wlocs = []
    for k in range(M):
        lo, hi = bounds[k], bounds[k + 1]
        wloc = (dst_s[lo:hi] - k * ndc) // WIN
        cnts[k] = np.bincount(wloc, minlength=nwin)
        wlocs.append(wloc)
    npad = np.maximum(((cnts.max(0) + P - 1) // P) * P, P)  # [nwin]
    offs = np.zeros(nwin + 1, np.int64)
    offs[1:] = np.cumsum(npad)
    S = int(offs[-1])
    T = S // P

    wrec = (1.0 / np.maximum(np.bincount(dst, minlength=n_dst), 1.0)).astype(F32)

    xT = np.zeros((M * P, S), F16)
    dstw = np.full((M, S), PAD_DST, F32)
    for k in range(M):
        lo, hi = bounds[k], bounds[k + 1]
        wloc = wlocs[k]
        runstart = np.zeros(nwin, np.int64)
        runstart[1:] = np.cumsum(cnts[k][:-1])
        n = hi - lo
        pos = offs[wloc] + (np.arange(n) - runstart[wloc])
        # Prelu(w*z) == w*Prelu(z) for w > 0 and z is linear in x, so the
        # 1/deg weight folds into the payload host-side
        xp = (emb32[src_s[lo:hi]] + sat_s[lo:hi].astype(F32) @ WsbInv[:4]
              + WsbInv[4]) * wrec[dst_s[lo:hi]][:, None]
        xT[k * P:(k + 1) * P, pos] = xp.astype(F16).T
        dstw[k, pos] = (dst_s[lo:hi] - k * ndc - wloc * WIN).astype(F32)
    dstT = np.ascontiguousarray(
        dstw.reshape(M, T, P).transpose(0, 2, 1).reshape(M * P, T))
    return dict(nwin=nwin, npad=npad.tolist(), S=S, T=T,
                xT=xT, dstT=dstT)


def _node_prep(feats, emb16, ctx_ids, n_nodes, Np):
    """Per-core node-phase arrays, stacked core-major."""
    nn = n_nodes // M
    nf = feats.shape[1]
    featsT = np.zeros((M * (nf + 1), Np), F16)
    embT = np.zeros((M * P, Np), F16)
    ohuT = np.zeros((M * 64, Np), F16)
    for k in range(M):
        fs, es, cs_ = (a[k * nn:(k + 1) * nn] for a in (feats, emb16, ctx_ids))
        featsT[k * (nf + 1):k * (nf + 1) + nf, :nn] = fs.T.astype(F16)
        featsT[k * (nf + 1) + nf, :nn] = 1.0
        embT[k * P:(k + 1) * P, :nn] = es.T
        ohuT[k * 64 + cs_, np.arange(nn)] = 1.0
    return featsT, embT, ohuT


def _build_program(meta):
    import concourse.mybir as mybir
    import concourse.tile as tile
    from concourse import bacc

    f16, f32 = mybir.dt.float16, mybir.dt.float32

    nc = bacc.Bacc("TRN2", target_bir_lowering=False, debug=False, num_devices=1)
    io = {}

    def dram(name, shape, dt, kind="ExternalInput"):
        io[name] = nc.dram_tensor(name, list(shape), dt, kind=kind)
        return io[name]

    for side in ("A", "B"):
        pl = meta[side]
        dram(f"xT{side}", [P, pl["S"]], f16)
        dram(f"dstT{side}", [P, pl["T"]], f32)
        dram(f"Wemb{side}", [P, P], f16)
    for sd in ("C", "V"):
        Np = meta[f"Np{sd}"]
        dram(f"featsT{sd}", [17, Np], f16)
        dram(f"embT{sd}", [P, Np], f16)
        dram(f"ohuT{sd}", [64, Np], f16)
        dram(f"Wf{sd}", [17, P], f16)
        dram(f"Wh{sd}", [P, P], f16)
        dram(f"We{sd}", [P, P], f16)
        dram(f"ctxproj{sd}", [64, P], f16)
        dram(f"out{sd}", [P, Np], f16, kind="ExternalOutput")

    maxslot = max(max(meta["A"]["npad"]), max(meta["B"]["npad"]))

    with tile.TileContext(nc) as tc:
        with tc.tile_pool(name="const", bufs=1) as cpool, \
             tc.tile_pool(name="xs", bufs=3) as xpool, \
             tc.tile_pool(name="work", bufs=5) as wpool, \
             tc.tile_pool(name="oh", bufs=12) as opool, \
             tc.tile_pool(name="psA", bufs=2, space="PSUM") as psA, \
             tc.tile_pool(name="psH", bufs=2, space="PSUM") as psH, \
             tc.tile_pool(name="psN", bufs=2, space="PSUM") as psN:

            iota_i = cpool.tile([P, WIN], mybir.dt.int32)
            nc.gpsimd.iota(iota_i[:], pattern=[[1, WIN]], base=0,
                           channel_multiplier=0)
            iota16 = cpool.tile([P, WIN], f16)
            nc.vector.tensor_copy(iota16[:], iota_i[:])

            wt = {}
            for nm in ("WembA", "WembB",
                       "WfC", "WhC", "WeC", "ctxprojC",
                       "WfV", "WhV", "WeV", "ctxprojV"):
                t = cpool.tile(list(io[nm].shape), f16, tag=nm)
                nc.sync.dma_start(t[:], io[nm][:])
                wt[nm] = t

            for side, sd in (("A", "C"), ("B", "V")):
                pl = meta[side]
                nwin = pl["nwin"]
                dstall = cpool.tile([P, pl["T"]], f32, tag=f"dst{side}")
                nc.sync.dma_start(dstall[:], io[f"dstT{side}"][:])

                off = 0
                tbase = 0
                hTpair = None
                for w in range(nwin):
                    slots = pl["npad"][w]
                    nt = slots // P
                    xw = xpool.tile([P, maxslot], f16, tag="xw")
                    nc.sync.dma_start(xw[:, :slots],
                                      io[f"xT{side}"][:, off:off + slots])
                    hps = psH.tile([P, WIN], f32, tag="hps")
                    for j0 in range(0, nt, GRP):
                        gn = min(GRP, nt - j0)
                        mps = psA.tile([P, GRP * P], f32, tag="mps")
                        msb = wpool.tile([P, GRP * P], f16, tag="msb")
                        ohws = []
                        for jj in range(gn):
                            t = tbase + j0 + jj
                            nc.tensor.matmul(mps[:, jj * P:(jj + 1) * P],
                                             lhsT=xw[:, (j0 + jj) * P:
                                                     (j0 + jj + 1) * P],
                                             rhs=wt[f"Wemb{side}"][:],
                                             start=True, stop=True)
                            # pure one-hot of dst (1/deg folded into x'';
                            # pad slots carry dst sentinel -> zero column)
                            ohw = opool.tile([P, WIN], f16, tag="ohw")
                            nc.vector.tensor_single_scalar(
                                out=ohw[:], in_=iota16[:],
                                scalar=dstall[:, t:t + 1],
                                op=mybir.AluOpType.is_equal)
                            ohws.append(ohw)
                        nc.scalar.activation(
                            msb[:, :gn * P], mps[:, :gn * P],
                            mybir.ActivationFunctionType.Prelu, alpha=0.1)
                        for jj in range(gn):
                            j = j0 + jj
                            nc.tensor.matmul(hps[:],
                                             lhsT=msb[:, jj * P:(jj + 1) * P],
                                             rhs=ohws[jj][:],
                                             start=(j == 0), stop=(j == nt - 1),
                                             skip_group_check=True)
                    off += slots
                    tbase += nt

                    if hTpair is None:
                        hTpair = wpool.tile([P, 2 * WIN], f16, tag="hT")
                    nc.scalar.copy(
                        hTpair[:, (w % 2) * WIN:(w % 2) * WIN + WIN], hps[:])

                    if w % 2 == 0 and w != nwin - 1:
                        continue
                    # node MLP for the last 1-2 windows (N = 512 or 256)
                    nw = WIN if w % 2 == 0 else 2 * WIN
                    cga = (w + 1) * WIN - nw
                    featsl = wpool.tile([17, 2 * WIN], f16, tag="featsl")
                    nc.sync.dma_start(featsl[:, :nw],
                                      io[f"featsT{sd}"][:, cga:cga + nw])
                    embl = wpool.tile([P, 2 * WIN], f16, tag="embl")
                    nc.sync.dma_start(embl[:, :nw],
                                      io[f"embT{sd}"][:, cga:cga + nw])
                    ohul = wpool.tile([64, 2 * WIN], f16, tag="ohul")
                    nc.sync.dma_start(ohul[:, :nw],
                                      io[f"ohuT{sd}"][:, cga:cga + nw])
                    nps = psN.tile([P, 2 * WIN], f32, tag="nps")
                    nc.tensor.matmul(nps[:, :nw], lhsT=wt[f"Wf{sd}"][:],
                                     rhs=featsl[:, :nw], start=True, stop=False)
                    nc.tensor.matmul(nps[:, :nw], lhsT=wt[f"Wh{sd}"][:],
                                     rhs=hTpair[:, :nw],
                                     start=False, stop=False)
                    nc.tensor.matmul(nps[:, :nw], lhsT=wt[f"ctxproj{sd}"][:],
                                     rhs=ohul[:, :nw], start=False, stop=False)
                    nc.tensor.matmul(nps[:, :nw], lhsT=wt[f"We{sd}"][:],
                                     rhs=embl[:, :nw], start=False, stop=True)
                    nsb = wpool.tile([P, 2 * WIN], f16, tag="nsb")
                    nc.scalar.activation(nsb[:, :nw], nps[:, :nw],
                                         mybir.ActivationFunctionType.Prelu,
                                         alpha=0.1)
                    nc.sync.dma_start(io[f"out{sd}"][:, cga:cga + nw],
                                      nsb[:, :nw])
                    hTpair = None
    nc.compile()
    return nc


_spmd_state = {}

REPLICATED = ("WembA", "WembB",
              "WfC", "WhC", "WeC", "ctxprojC",
              "WfV", "WhV", "WeV", "ctxprojV")


def _run_spmd(nc, stacked_map, repl_map):
    """One shard_map dispatch running the identical program on all 8 cores.

    stacked_map[nm] is the core-major stacked array [M*rows, ...]; repl_map
    holds the small replicated weights."""
    import concourse.mybir as mybir
    import jax
    from concourse.bass2jax import (_bass_exec_p, install_neuronx_cc_hook,
                                    partition_id_tensor)
    from jax.experimental.shard_map import shard_map
    from jax.sharding import Mesh, NamedSharding, PartitionSpec

    install_neuronx_cc_hook()
    partition_name = nc.partition_id_tensor.name if nc.partition_id_tensor else None
    in_names, out_names, out_avals, zero_shapes = [], [], [], []
    for alloc in nc.m.functions[0].allocations:
        if not isinstance(alloc, mybir.MemoryLocationSet):
            continue
        name = alloc.memorylocations[0].name
        if alloc.kind == "ExternalInput":
            if name != partition_name:
                in_names.append(name)
        elif alloc.kind == "ExternalOutput":
            shape = tuple(alloc.tensor_shape)
            dtype = mybir.dt.np(alloc.dtype)
            out_names.append(name)
            out_avals.append(jax.core.ShapedArray(shape, dtype))
            zero_shapes.append((shape, dtype))
    n_params = len(in_names)
    n_outs = len(out_names)
    all_names = list(in_names) + list(out_names)
    if partition_name is not None:
        all_names.append(partition_name)
    donate = tuple(range(n_params, n_params + n_outs))

    def _body(*args):
        operands = list(args)
        if partition_name is not None:
            operands.append(partition_id_tensor())
        return tuple(_bass_exec_p.bind(
            *operands, out_avals=tuple(out_avals), in_names=tuple(all_names),
            out_names=tuple(out_names), lowering_input_output_aliases=(),
            sim_require_finite=True, sim_require_nnan=True, nc=nc))

    devices = jax.devices()[:M]
    mesh = Mesh(np.asarray(devices), ("core",))
    in_specs = tuple(
        PartitionSpec() if nm in REPLICATED else PartitionSpec("core")
        for nm in in_names) + (PartitionSpec("core"),) * n_outs
    out_specs = (PartitionSpec("core"),) * n_outs
    sharded = jax.jit(
        shard_map(_body, mesh=mesh, in_specs=in_specs, out_specs=out_specs,
                  check_rep=False),
        donate_argnums=donate, keep_unused=True)

    sh_core = NamedSharding(mesh, PartitionSpec("core"))
    sh_repl = NamedSharding(mesh, PartitionSpec())

    d_ins = []
    for nm in in_names:
        if nm in REPLICATED:
            d_ins.append(jax.device_put(repl_map[nm], sh_repl))
        else:
            d_ins.append(jax.device_put(stacked_map[nm], sh_core))
    d_zeros = [jax.device_put(np.zeros((M * s[0], *s[1:]), dt), sh_core)
               for (s, dt) in zero_shapes]

    outs = sharded(*d_ins, *d_zeros)
    host = {nm: np.asarray(o) for nm, o in zip(out_names, outs)}

    _spmd_state.clear()
    _spmd_state.update(fn=sharded, d_ins=d_ins, outs=list(outs),
                       out_names=out_names, sh_core=sh_core,
                       zero_shapes=zero_shapes, nc=nc)
    return host


def _redispatch():
    """Re-run the compiled program once (fresh zero output buffers)."""
    import jax
    st = _spmd_state
    d_zeros = [jax.device_put(np.zeros((M * s[0], *s[1:]), dt), st["sh_core"])
               for (s, dt) in st["zero_shapes"]]
    outs = st["fn"](*st["d_ins"], *d_zeros)
    st["outs"] = list(outs)
    return {nm: np.asarray(o) for nm, o in zip(st["out_names"], outs)}


def _assemble(host_out, n_nodes, Np):
    """[M*P, Np] core-major d-major output -> [n_nodes, 128] f32."""
    nn = n_nodes // M
    parts = [host_out[k * P:(k + 1) * P, :nn] for k in range(M)]
    return np.concatenate(parts, axis=1).T.astype(F32)


def _segmean(x, ids, n):
    order = np.argsort(ids, kind="stable")
    xs = x[order]
    ids_s = ids[order]
    starts = np.searchsorted(ids_s, np.arange(n))
    cnt = np.bincount(ids, minlength=n).astype(F32)
    sums = np.zeros((n, x.shape[1]), F32)
    nz = cnt > 0
    if nz.any():
        sums[nz] = np.add.reduceat(xs, starts[nz], axis=0)
    return sums / np.maximum(cnt, 1.0)[:, None]


def _spot_check(new_clause, new_var, inp, a_src, a_dst, c_src, c_dst,
                clause_ctx, var_ctx, n_sample=48, seed=7):
    """Exact host recomputation of a few output nodes.  Clean device runs
    differ by <2e-2 absmax (f16 + sat-fold path); silent corruption differs
    by ~2.5 — threshold 0.2 separates them cleanly."""
    rng = np.random.default_rng(seed)
    lrelu = lambda x: np.where(x >= 0, x, 0.1 * x)
    ok = True
    for (emb_src, src, dst, sat, Wm, bm, feats, Wn, bn, ctx_ids, emb_self,
         got_all) in (
            (inp["var_emb"], a_src, a_dst, inp["edge_sat_vc"], inp["W_vc"],
             inp["b_vc"], inp["clause_feats"], inp["W_c"], inp["b_c"],
             clause_ctx, inp["clause_emb"], new_clause),
            (inp["clause_emb"], c_src, c_dst, inp["edge_sat_cv"], inp["W_cv"],
             inp["b_cv"], inp["var_feats"], inp["W_v"], inp["b_v"],
             var_ctx, inp["var_emb"], new_var)):
        n_nodes = feats.shape[0]
        sample = rng.choice(n_nodes, size=min(n_sample, n_nodes), replace=False)
        sel = np.isin(dst, sample)
        es, ds = src[sel], dst[sel]
        m = lrelu(np.concatenate([sat[sel].astype(F32),
                                  emb_src[es].astype(F32)], 1)
                  @ Wm.astype(F32) + bm.astype(F32))
        h = np.zeros((n_nodes, 128), F32)
        np.add.at(h, ds, m)
        cnt = np.bincount(dst, minlength=n_nodes).astype(F32)
        h = h / np.maximum(cnt, 1.0)[:, None]
        ctx_e = inp["ctx_emb"][ctx_ids[sample]].astype(F32)
        z = np.concatenate([feats[sample].astype(F32), h[sample], ctx_e,
                            emb_self[sample].astype(F32)], 1) \
            @ Wn.astype(F32) + bn.astype(F32)
        ref = lrelu(z)
        if np.abs(got_all[sample] - ref).max() > 0.2:
            ok = False
    return ok


def kernel(**inputs):
    inp = {k: np.asarray(v) for k, v in inputs.items()}
    var_emb, clause_emb, ctx_emb = inp["var_emb"], inp["clause_emb"], inp["ctx_emb"]
    nv, ncl, nu = var_emb.shape[0], clause_emb.shape[0], ctx_emb.shape[0]

    a_src = inp["assigns_src"].astype(np.int64)
    a_dst = inp["assigns_dst"].astype(np.int64)
    c_src = inp["contains_src"].astype(np.int64)
    c_dst = inp["contains_dst"].astype(np.int64)
    var_ctx = inp["var_ctx"].astype(np.int64)
    clause_ctx = inp["clause_ctx"].astype(np.int64)

    # fold [sat, 1] @ [W_sat; b] through W_emb^{-1} into the gathered x rows
    W_vc, b_vc = inp["W_vc"].astype(F32), inp["b_vc"].astype(F32)
    W_cv, b_cv = inp["W_cv"].astype(F32), inp["b_cv"].astype(F32)
    WsbInvA = (np.vstack([W_vc[:4], b_vc[None, :]]).astype(np.float64)
               @ np.linalg.inv(W_vc[4:132].astype(np.float64))).astype(F32)
    WsbInvB = (np.vstack([W_cv[:4], b_cv[None, :]]).astype(np.float64)
               @ np.linalg.inv(W_cv[4:132].astype(np.float64))).astype(F32)

    planA = _side_prep(a_src, a_dst, inp["edge_sat_vc"],
                       var_emb.astype(F32), WsbInvA, ncl)
    planB = _side_prep(c_src, c_dst, inp["edge_sat_cv"],
                       clause_emb.astype(F32), WsbInvB, nv)

    NpC = planA["nwin"] * WIN
    NpV = planB["nwin"] * WIN
    emb16V = var_emb.astype(F16)
    emb16C = clause_emb.astype(F16)
    featsTC, embTC, ohuTC = _node_prep(inp["clause_feats"], emb16C,
                                       clause_ctx, ncl, NpC)
    featsTV, embTV, ohuTV = _node_prep(inp["var_feats"], emb16V,
                                       var_ctx, nv, NpV)

    def node_w(Wn, bn):
        Wn, bn = Wn.astype(F32), bn.astype(F32)
        nf = Wn.shape[0] - 3 * 128
        Wf = np.vstack([Wn[:nf], bn[None, :]]).astype(F16)
        Wh = np.ascontiguousarray(Wn[nf:nf + 128]).astype(F16)
        ctxproj = (ctx_emb.astype(F32) @ Wn[nf + 128:nf + 256]).astype(F16)
        We = np.ascontiguousarray(Wn[nf + 256:nf + 384]).astype(F16)
        return Wf, Wh, ctxproj, We

    WfC, WhC, ctxprojC, WeC = node_w(inp["W_c"], inp["b_c"])
    WfV, WhV, ctxprojV, WeV = node_w(inp["W_v"], inp["b_v"])

    repl_map = dict(
        WembA=np.ascontiguousarray(W_vc[4:132]).astype(F16),
        WembB=np.ascontiguousarray(W_cv[4:132]).astype(F16),
        WfC=WfC, WhC=WhC, WeC=WeC, ctxprojC=ctxprojC,
        WfV=WfV, WhV=WhV, WeV=WeV, ctxprojV=ctxprojV,
    )
    stacked_map = dict(
        xTA=planA["xT"], dstTA=planA["dstT"],
        xTB=planB["xT"], dstTB=planB["dstT"],
        featsTC=featsTC, embTC=embTC, ohuTC=ohuTC,
        featsTV=featsTV, embTV=embTV, ohuTV=ohuTV,
    )

    meta = dict(
        A=dict(nwin=planA["nwin"], npad=planA["npad"], S=planA["S"], T=planA["T"]),
        B=dict(nwin=planB["nwin"], npad=planB["npad"], S=planB["S"], T=planB["T"]),
        NpC=NpC, NpV=NpV)
    nc = _build_program(meta)
    host = _run_spmd(nc, stacked_map, repl_map)

    new_clause = _assemble(host["outC"], ncl, NpC)
    new_var = _assemble(host["outV"], nv, NpV)
    # guard against rare silent corruption on the terminal
    for _ in range(2):
        if _spot_check(new_clause, new_var, inp, a_src, a_dst, c_src, c_dst,
                       clause_ctx, var_ctx):
            break
        host = _redispatch()
        new_clause = _assemble(host["outC"], ncl, NpC)
        new_var = _assemble(host["outV"], nv, NpV)

    # Phase 3 on host
    c_ctx = _segmean(new_clause, clause_ctx, nu)
    v_ctx = _segmean(new_var, var_ctx, nu)
    zu = np.concatenate([inp["ctx_feats"].astype(F32), c_ctx, v_ctx,
                         ctx_emb.astype(F32)], 1) @ inp["W_u"].astype(F32) \
        + inp["b_u"].astype(F32)
    new_ctx = np.where(zu >= 0, zu, 0.1 * zu).astype(F32)

    return np.concatenate([new_clause, new_var, new_ctx], 0).astype(F32)
